# revision 57
# baseline (speedup 1.0000x reference)
"""Trainium2 Bass kernel for BasicAttention (B=16, C=1024, Q=128, H=768).

Strategy
--------
Data-parallel over batch: 8 NeuronCores x 2 batches each. No collectives.

Per batch (X = context[b] [C,H], Qm = query[b] [Q,H]):
  qry   = Qm @ Wq^T + bq                      [Q,H]
  G     = (qry * w_att) @ Wc                  [Q,H]   (fused-projection trick)
  r     = Qm @ (Wq^T (w_att*bc))              [Q]     (const part softmax-invariant)
  sim   = X @ G^T + r (+ b_att, dropped: softmax/max-softmax shift-invariant)
  ctx   = X @ Wc^T + bc                       [C,H]
  alpha = softmax_q(sim);  a = (alpha*masks) @ qry
  beta  = softmax_c(max_q sim) * cmask;  b = beta @ ctx
  out   = [ctx, a, ctx*a, ctx*b]              [C,4H]

All big operands are fp16 on both sides: inputs are cast on the host and the
entire output is written fp16 and upcast to fp32 on the host after the
gather (output is 3/4 of HBM traffic; fp16 adds <=2^-11 absmax-rel error).
ctx lives in SBUF as fp16 so the c/d muls and a*ctx run in the DVE 16-bit
mode and the beta matmuls take fp16 operands directly.

exp(sim^T) comes straight out of the sim matmul via one ACT pass (r folded
in as the per-partition bias; |sim|<~3 so the softmax max-shift is
droppable) into fp16 expsimT [q,c] which feeds the a-matmul lhsT. Softmax
stats are fp16 PE transposes of expsimT tiles + DVE reductions.

The query-side projection is computed ONCE in transposed form (qryT[m] =
(Wq @ Qm^T)[m] via 36 128^3 matmuls); qwT and qryT+bq are two ACT passes
over the same PSUM tiles, and natural-layout qmm comes from 6 fp16 PE
transposes. r is a 6-step matvec against host-precomputed v = Wq^T(w_att*bc).

Scheduling is driven by the measured HW behavior:
 - HAM clock gate: PE defaults to 1.2 GHz and only reaches 2.4 GHz after
   ~3.4us of sustained activity -> a 10-matmul junk warm-up burst at t=0
   flips it early and the early real matmuls run 2x faster.
 - Each engine owns one hardware DGE queue (~110-160 GB/s each): inputs and
   outputs are spread across the sync/scalar/tensor/vector/gpsimd queues so
   no single ring serializes the traffic.
 - The last batch runs two-phase: all ctx tiles + sim/stats/beta first
   (PE-heavy), then a/c/d streamed per tile (DVE/ACT/DMA-heavy) so the
   d-quarter is not a serial tail.
"""

import os

import numpy as np

import concourse.bass as bass
import concourse.tile as tile
from concourse import bacc, bass_isa, mybir
from concourse.bass_utils import run_bass_kernel_spmd

F32 = mybir.dt.float32
F32R = mybir.dt.float32r
F16 = mybir.dt.float16
AX = mybir.AxisListType.X
EXP = mybir.ActivationFunctionType.Exp
IDENT = mybir.ActivationFunctionType.Identity

B, C, Q, H = 16, 1024, 128, 768
NC = 8
BL = B // NC          # batches per core
HT = H // 128         # 6 h-chunks
CT = C // 128         # 8 c-tiles
NSPLIT = ((0, 512), (512, 256))  # free-dim split respecting PSUM banks

_CACHED = None


def _build():
    nc = bacc.Bacc("TRN2", debug=False)

    # xTm[p, t, j, q] = X[t*128+q, j*128+p]: c-tile-major swizzled X^T so each
    # [128,128] (t,j) block is one matmul operand and quarters stream in early.
    xTm_in = nc.dram_tensor("xTm_in", (BL, 128, CT, HT, 128), F16, kind="ExternalInput")
    qT_in = nc.dram_tensor("qT_in", (128, BL * HT * Q), F16, kind="ExternalInput")
    wcT_d = nc.dram_tensor("wcT", (128, HT * H), F16, kind="ExternalInput")
    wc_d = nc.dram_tensor("wc", (128, HT * H), F16, kind="ExternalInput")
    wqT_d = nc.dram_tensor("wqT", (128, HT * H), F16, kind="ExternalInput")
    # const blob cols: wac[0:6] cm[6:22] qm[22:24] bcb[24:792] bqw[792:798]
    # bqT[798:804]
    cb_d = nc.dram_tensor("cblob", (128, 805), F32, kind="ExternalInput")
    # fp16 blob: identity[0:128] ones[128] vb[129:135]
    i16_d = nc.dram_tensor("iden16", (128, 135), F16, kind="ExternalInput")
    one_d = nc.dram_tensor("ones32", (1, 128), F32, kind="ExternalInput")
    out_d = nc.dram_tensor("out", (BL, C, 4 * H), F16, kind="ExternalOutput")

    with tile.TileContext(nc) as tc:
        with (
            tc.tile_pool(name="const", bufs=1) as cpool,
            tc.tile_pool(name="xt", bufs=2) as xtpool,
            tc.tile_pool(name="bigp", bufs=2) as bigpool,
            tc.tile_pool(name="qside", bufs=1) as qpool,
            tc.tile_pool(name="qside2", bufs=2) as q2pool,
            tc.tile_pool(name="ev", bufs=2) as evpool,
            tc.tile_pool(name="ev3", bufs=3) as ev3pool,
            tc.tile_pool(name="stat", bufs=1) as stpool,
            tc.tile_pool(name="ps768", bufs=2, space="PSUM") as ps768,
            tc.tile_pool(name="ps512", bufs=2, space="PSUM") as ps512,
            tc.tile_pool(name="pst", bufs=2, space="PSUM") as pst,
        ):
            # ---- constants / weights (once per core) ----
            wcT = cpool.tile([128, HT * H], F16, tag="wcT")   # block j: WcT[128j:128j+128, :]
            wqT = cpool.tile([128, HT * H], F16, tag="wqT")
            wcn = cpool.tile([128, HT * H], F16, tag="wcn")   # Wc natural (built on-chip)
            cb = cpool.tile([128, 805], F32, tag="cb")
            onesc = cb[:, 804:805]
            wac = cb[:, 0:6]
            cm = cb[:, 6:22]
            qm = cb[:, 22:24]
            bcb = cb[:, 24:24 + H]
            bqw = cb[:, 792:798]
            bqT = cb[:, 798:804]
            iden16 = cpool.tile([128, 135], F16, tag="iden16")
            i128 = iden16[:, 0:128]
            vb = iden16[:, 129:135]
            ones32 = cpool.tile([1, 128], F32R, tag="ones32")
            qT = {}
            xT = {}
            qTb = qpool.tile([128, BL * H], F16, tag="qTb")
            for lb in range(BL):
                qT[lb] = qTb[:, lb * H:(lb + 1) * H]
                xT[lb] = xtpool.tile([128, CT, HT, 128], F16, tag="xT", name=f"xT{lb}")

            # ---- input DMA: 2 HW DGE rings (sync/scalar, ~110-160 GB/s per
            # queue) carry the longest dependency chain's operands first
            # (wqT -> qpost -> G -> sim); the gpsimd software DGE (~160 GB/s,
            # measured) is a third queue for wcn/wcT_h1/batch-1 operands ----
            HH = HT * H // 2
            nc.sync.dma_start(iden16[:], i16_d.ap()[:, :])
            nc.sync.dma_start(ones32[:], one_d.ap()[:, :].bitcast(F32R))
            nc.sync.dma_start(wqT[:, HH:2 * HH], wqT_d.ap()[:, HH:2 * HH])
            nc.sync.dma_start(cb[:], cb_d.ap()[:, :])
            nc.sync.dma_start(xT[0][:, 0:2, :, :], xTm_in.ap()[0, :, 0:2, :, :])
            nc.sync.dma_start(xT[0][:, 4:6, :, :], xTm_in.ap()[0, :, 4:6, :, :])
            nc.sync.dma_start(xT[0][:, 6:8, :, :], xTm_in.ap()[0, :, 6:8, :, :])
            nc.scalar.dma_start(wqT[:, 0:HH], wqT_d.ap()[:, 0:HH])
            nc.scalar.dma_start(qTb[:, 0:H], qT_in.ap()[:, 0:H])
            nc.scalar.dma_start(wcT[:, 0:HH], wcT_d.ap()[:, 0:HH])
            nc.gpsimd.dma_start(wcn[:], wc_d.ap()[:, :])
            nc.gpsimd.dma_start(wcT[:, HH:2 * HH], wcT_d.ap()[:, HH:2 * HH])
            nc.gpsimd.dma_start(xT[0][:, 2:4, :, :], xTm_in.ap()[0, :, 2:4, :, :])
            nc.gpsimd.dma_start(qTb[:, H:2 * H], qT_in.ap()[:, H:2 * H])
            nc.gpsimd.dma_start(xT[1][:, 0:4, :, :], xTm_in.ap()[1, :, 0:4, :, :])
            nc.gpsimd.dma_start(xT[1][:, 4:8, :, :], xTm_in.ap()[1, :, 4:8, :, :])

            qmm = {}
            gT = {}
            r_sb = {}
            qwT = {}
            g_ps = {}

            # PE warm-up: dense junk matmuls while the first inputs stream in.
            # HAM needs ~3.4us of sustained PE activity to release the 1.2->2.4
            # GHz clock gate; this burst flips it before the real work starts.
            junk = qpool.tile([128, 512], F16, tag="junk")
            nc.gpsimd.memset(junk[:], 1.0)
            # NOTE: keep warm-up matmuls in pst with unique names -- tiles in
            # a shared-tag pool with no reader get dead-code-eliminated.
            for wi in range(24):
                jp = pst.tile([128, 512], F32, tag="tp", name=f"warm{wi}")
                nc.tensor.matmul(jp[:], junk[:, 0:128], junk[:], start=True, stop=True)

            def qpost(lb, split=False):
                # qryT[m-block] = (Wq @ Qm^T)[m] once; two ACT reads of the
                # same PSUM tile give qwT (scaled) and qnT (plain, +bq).
                # (One accumulation group per PSUM bank region at a time --
                # multiple open groups in one bank corrupt accumulation.)
                qwT[lb] = qpool.tile([128, H], F16, tag="qwT", name=f"qwT{lb}")
                qnT = q2pool.tile([128, H], F16, tag="qnT", name=f"qnT{lb}")

                # r[q] = (Qm @ v)[q], v = Wq^T (w_att*bc) precomputed on host
                r_ps = pst.tile([128, 1], F32, tag="tp", name=f"rps{lb}")
                for j in range(HT):
                    nc.tensor.matmul(r_ps[:], qT[lb][:, j * 128:(j + 1) * 128],
                                     vb[:, j:j + 1],
                                     start=(j == 0), stop=(j == HT - 1))
                r_sb[lb] = stpool.tile([128, 1], F32, tag=f"r_sb{lb}", name=f"r_sb{lb}")
                nc.scalar.copy(r_sb[lb][:], r_ps[:])

                if split:
                    # bridge the wqT-landing window with warm-keeper matmuls
                    for wi in range(8):
                        jp = pst.tile([128, 512], F32, tag="tp",
                                      name=f"warmq{wi}")
                        nc.tensor.matmul(jp[:], junk[:, 0:128], junk[:],
                                         start=True, stop=True)
                for m in range(HT):
                    qt_ps = pst.tile([128, 128], F32, tag="tp", name=f"qt{lb}{m}")
                    for j in range(HT):
                        nc.tensor.matmul(qt_ps[:],
                                         wqT[:, j * H + m * 128: j * H + (m + 1) * 128],
                                         qT[lb][:, j * 128:(j + 1) * 128],
                                         start=(j == 0), stop=(j == HT - 1))
                    nc.scalar.activation(qwT[lb][:, m * 128:(m + 1) * 128], qt_ps[:],
                                         IDENT, scale=wac[:, m:m + 1],
                                         bias=bqw[:, m:m + 1])
                    nc.scalar.activation(qnT[:, m * 128:(m + 1) * 128], qt_ps[:],
                                         IDENT, bias=bqT[:, m:m + 1])

                # natural-layout qmm = qry*qmask via fp16 PE transposes
                qmm[lb] = q2pool.tile([128, H], F16, tag="qmm", name=f"qmm{lb}")
                for m in range(HT):
                    tp = pst.tile([128, 128], F16, tag="tp", name=f"qx{lb}{m}")
                    nc.tensor.transpose(tp[:], qnT[:, m * 128:(m + 1) * 128], i128)
                    nc.vector.tensor_scalar_mul(qmm[lb][:, m * 128:(m + 1) * 128],
                                                tp[:], qm[:, lb:lb + 1])

            def G_mm(lb):
                g_ps[lb] = ps768.tile([128, H], F32, tag="mm768", name=f"g_ps{lb}")
                for j in range(HT):
                    for (n0, nw) in NSPLIT:
                        nc.tensor.matmul(g_ps[lb][:, n0:n0 + nw],
                                         qwT[lb][:, j * 128:(j + 1) * 128],
                                         wcn[:, j * H + n0: j * H + n0 + nw],
                                         start=(j == 0), stop=(j == HT - 1))

            def gpost(lb):
                g_sb = qpool.tile([128, H], F16, tag="g_sb", name=f"g_sb{lb}")
                nc.scalar.copy(g_sb[:], g_ps[lb][:])
                gT[lb] = q2pool.tile([128, H], F16, tag="gT", name=f"gT{lb}")
                for j in range(HT):
                    tp = pst.tile([128, 128], F16, tag="tp", name=f"tg{lb}{j}")
                    nc.tensor.transpose(tp[:], g_sb[:, j * 128:(j + 1) * 128], i128)
                    nc.vector.tensor_copy(gT[lb][:, j * 128:(j + 1) * 128], tp[:])

            # ---- context phases ----
            pending_d = []
            bb_sb = {}
            for lb in range(BL):
                ctx_all = bigpool.tile([128, CT * H], F16, tag="big", name=f"ctx{lb}")
                expsT = q2pool.tile([128, C], F16, tag="expsT", name=f"expsT{lb}")
                rsum = stpool.tile([128, CT], F32, tag=f"rsum{lb}", name=f"rsum{lb}")
                rcp = stpool.tile([128, CT], F32, tag=f"rcp{lb}", name=f"rcp{lb}")
                rscm = stpool.tile([128, CT], F32, tag=f"rscm{lb}", name=f"rscm{lb}")
                w8 = stpool.tile([128, CT], F32, tag=f"w8{lb}", name=f"w8{lb}")
                wm8 = stpool.tile([128, CT], F16, tag=f"wm8{lb}", name=f"wm8{lb}")
                b_acc = stpool.tile([1, H], F32R, tag=f"bacc{lb}", name=f"bacc{lb}")

                def simB(u, lb=lb, expsT=expsT):
                    """exp(sim^T) half u: [q, 512c] fp16, r added as ACT bias."""
                    st_ps = ps512.tile([128, 512], F32, tag="mm512")
                    for j in range(HT):
                        nc.tensor.matmul(st_ps[:],
                                         gT[lb][:, j * 128:(j + 1) * 128],
                                         xT[lb][:, u * 4:(u + 1) * 4, j, :],
                                         start=(j == 0), stop=(j == HT - 1))
                    nc.scalar.activation(expsT[:, u * 512:(u + 1) * 512], st_ps[:],
                                         EXP, bias=r_sb[lb][:])

                def stats(t, lb=lb, expsT=expsT, rsum=rsum, rcp=rcp, rscm=rscm,
                          w8=w8, wm8=wm8):
                    """fp16 transpose of an expsimT tile -> row stats on DVE."""
                    eT_ps = pst.tile([128, 128], F16, tag="tp", name=f"eT{lb}_{t}")
                    nc.tensor.transpose(eT_ps[:], expsT[:, t * 128:(t + 1) * 128],
                                        i128)
                    nc.vector.reduce_max(w8[:, t:t + 1], eT_ps[:], axis=AX)
                    nc.vector.reduce_sum(rsum[:, t:t + 1], eT_ps[:], axis=AX)
                    nc.vector.tensor_mul(wm8[:, t:t + 1], w8[:, t:t + 1],
                                         cm[:, lb * CT + t: lb * CT + t + 1])
                    nc.vector.reciprocal(rcp[:, t:t + 1], rsum[:, t:t + 1])
                    nc.scalar.mul(rscm[:, t:t + 1], rcp[:, t:t + 1],
                                  cm[:, lb * CT + t: lb * CT + t + 1])

                def ctx_mm(t, lb=lb, ctx_all=ctx_all):
                    cx_ps = ps768.tile([128, H], F32, tag="mm768")
                    for j in range(HT):
                        for (n0, nw) in NSPLIT:
                            nc.tensor.matmul(cx_ps[:, n0:n0 + nw],
                                             xT[lb][:, t, j, :],
                                             wcT[:, j * H + n0: j * H + n0 + nw],
                                             start=(j == 0), stop=(j == HT - 1))
                    nc.vector.tensor_add(ctx_all[:, t * H:(t + 1) * H], cx_ps[:], bcb[:])
                    # batch 0's ctx rides the gpsimd SW queue (idle early);
                    # batch 1's goes on the HW rings so the SW queue is clear
                    # of large transfers before the tail d-stream lands on it
                    ring = (nc.gpsimd if lb == 0
                            else (nc.sync if t % 2 == 0 else nc.scalar))
                    ring.dma_start(out_d.ap()[lb, t * 128:(t + 1) * 128, 0:H],
                                   ctx_all[:, t * H:(t + 1) * H])

                def a_c(t, ceng=None, vnorm=False, lb=lb, ctx_all=ctx_all,
                        expsT=expsT, rscm=rscm):
                    a_ps = ps768.tile([128, H], F32, tag="mm768")
                    for (n0, nw) in NSPLIT:
                        nc.tensor.matmul(a_ps[:, n0:n0 + nw],
                                         expsT[:, t * 128:(t + 1) * 128],
                                         qmm[lb][:, n0:n0 + nw], start=True, stop=True)
                    stage = ev3pool.tile([128, 2 * H], F16, tag="stage")
                    if vnorm:
                        nc.vector.tensor_scalar_mul(stage[:, 0:H], a_ps[:],
                                                    rscm[:, t:t + 1])
                    else:
                        nc.scalar.mul(stage[:, 0:H], a_ps[:], rscm[:, t:t + 1])
                    (ceng or nc.vector).tensor_mul(
                        stage[:, H:2 * H], stage[:, 0:H],
                        ctx_all[:, t * H:(t + 1) * H])
                    (nc.sync if t % 2 == 0 else nc.scalar).dma_start(
                        out_d.ap()[lb, t * 128:(t + 1) * 128, H:3 * H], stage[:])

                def b_half(u, lb=lb, ctx_all=ctx_all, wm8=wm8, b_acc=b_acc):
                    """partial b = sum_c wm8[c]*ctx[c,:] over this half's 4 tiles"""
                    b5_ps = pst.tile([1, 512], F32, tag="tp", name=f"b5_{lb}{u}")
                    b2_ps = pst.tile([1, 256], F32, tag="tp", name=f"b2_{lb}{u}")
                    for tt in range(4):
                        t = u * 4 + tt
                        nc.tensor.matmul(b5_ps[:], wm8[:, t:t + 1],
                                         ctx_all[:, t * H: t * H + 512],
                                         start=(tt == 0), stop=(tt == 3))
                        nc.tensor.matmul(b2_ps[:], wm8[:, t:t + 1],
                                         ctx_all[:, t * H + 512: t * H + 768],
                                         start=(tt == 0), stop=(tt == 3))
                    if u == 0:
                        nc.vector.tensor_copy(b_acc[0:1, 0:512], b5_ps[:])
                        nc.vector.tensor_copy(b_acc[0:1, 512:H], b2_ps[:])
                    else:
                        nc.vector.tensor_add(b_acc[0:1, 0:512],
                                             b_acc[0:1, 0:512].bitcast(F32), b5_ps[:])
                        nc.vector.tensor_add(b_acc[0:1, 512:H],
                                             b_acc[0:1, 512:H].bitcast(F32), b2_ps[:])

                def beta_chain(lb=lb, w8=w8):
                    # 1/sum_c exp(q2c): free-axis partial, then the partition
                    # reduction + broadcast as two tiny PE matmuls (the gpsimd
                    # all-reduce sits in a busy FIFO and stalls the PE here)
                    sp = stpool.tile([128, 1], F32, tag=f"sp{lb}", name=f"sp{lb}")
                    nc.vector.reduce_sum(sp[:], w8[:, 0:CT], axis=AX)
                    sp16 = stpool.tile([128, 1], F16, tag=f"sp6{lb}",
                                       name=f"sp6{lb}")
                    nc.vector.tensor_copy(sp16[:], sp[:])
                    tot_ps = pst.tile([1, 1], F32, tag="tp", name=f"tot{lb}")
                    nc.tensor.matmul(tot_ps[:], sp16[:], iden16[:, 128:129],
                                     start=True, stop=True)
                    tot_sb = stpool.tile([1, 1], F16, tag=f"tot{lb}",
                                         name=f"tots{lb}")
                    nc.vector.tensor_copy(tot_sb[:], tot_ps[:])
                    spa_ps = pst.tile([128, 1], F32, tag="tp", name=f"spa{lb}")
                    nc.tensor.matmul(spa_ps[:], junk[0:1, 0:128], tot_sb[:],
                                     start=True, stop=True)
                    rs1 = stpool.tile([128, 1], F32, tag=f"rs1{lb}", name=f"rs1{lb}")
                    nc.vector.reciprocal(rs1[:], spa_ps[:])
                    return rs1

                def bb_chain(rs1, lb=lb, b_acc=b_acc):
                    # broadcast b to 128 partitions via K=1 ones outer product,
                    # folding the beta normalization into the PSUM->SBUF copy
                    bb_ps = ps768.tile([128, H], F32, tag="mm768", name=f"bb_ps{lb}")
                    for (n0, nw) in NSPLIT:
                        nc.tensor.matmul(bb_ps[:, n0:n0 + nw], ones32[:],
                                         b_acc[0:1, n0:n0 + nw], start=True, stop=True)
                    bb = evpool.tile([128, H], F16, tag="bb")
                    nc.scalar.mul(bb[:], bb_ps[:], rs1[:, 0:1])
                    return bb

                def emit_d(t, eng, ring, lb=lb, ctx_all=ctx_all):
                    d_sb = ev3pool.tile([128, H], F16, tag=("d_sb", "c_sb")[t % 2],
                                        name=f"d{lb}_{t}")
                    eng.tensor_mul(d_sb[:], ctx_all[:, t * H:(t + 1) * H],
                                   bb_sb[lb][:])
                    ring(out_d.ap()[lb, t * 128:(t + 1) * 128, 3 * H:4 * H], d_sb[:])

                def pop():
                    if pending_d:
                        pending_d.pop(0)()

                if lb == 0:
                    qpost(0, split=True)
                    G_mm(0)
                    gpost(0)
                    simB(0)
                    ctx_mm(0)
                    stats(0)
                    a_c(0)
                    ctx_mm(1)
                    stats(1)
                    a_c(1)
                    ctx_mm(2)
                    stats(2)
                    a_c(2)
                    ctx_mm(3)
                    stats(3)
                    a_c(3)
                    b_half(0)
                    simB(1)
                    stats(4)
                    ctx_mm(4)
                    a_c(4)
                    qpost(1)
                    stats(5)
                    ctx_mm(5)
                    a_c(5)
                    G_mm(1)
                    gpost(1)
                    stats(6)
                    ctx_mm(6)
                    a_c(6)
                    stats(CT - 1)
                    rs1 = beta_chain()
                    ctx_mm(CT - 1)
                    a_c(CT - 1)
                    b_half(1)
                    bb_sb[lb] = bb_chain(rs1)
                    pending_d = [
                        (lambda t=t, f=emit_d: f(t, nc.vector,
                                                 nc.gpsimd.dma_start))
                        for t in range(CT)
                    ]
                else:
                    # phase A: everything beta/sim/ctx (PE-heavy), lb=0's
                    # leftover d-quarters popped in to fill DVE/DMA slack
                    # (qpost/G/gpost for this batch were hoisted into lb=0)
                    simB(0)
                    ctx_mm(0)
                    pop()
                    ctx_mm(1)
                    pop()
                    ctx_mm(2)
                    pop()
                    stats(0)
                    ctx_mm(3)
                    pop()
                    stats(1)
                    a_c(0)
                    stats(2)
                    a_c(1, vnorm=True)
                    stats(3)
                    ctx_mm(4)
                    pop()
                    b_half(0)
                    simB(1)
                    a_c(2)
                    ctx_mm(5)
                    pop()
                    stats(4)
                    a_c(3, vnorm=True)
                    ctx_mm(6)
                    pop()
                    stats(5)
                    stats(6)
                    ctx_mm(7)
                    pop()
                    stats(7)
                    rs1 = beta_chain()
                    b_half(1)
                    # phase B: the remaining a/c plus the d stream (DVE/ACT/
                    # DMA-heavy); the first a_c's overlap the beta/bb chain.
                    a_c(4)
                    pop()
                    a_c(5)
                    bb_sb[lb] = bb_chain(rs1)
                    a_c(6)
                    emit_d(0, nc.vector, nc.gpsimd.dma_start)
                    a_c(7)
                    emit_d(1, nc.vector, nc.sync.dma_start)
                    emit_d(2, nc.gpsimd, nc.gpsimd.dma_start)
                    emit_d(3, nc.vector, nc.sync.dma_start)
                    emit_d(4, nc.vector, nc.gpsimd.dma_start)
                    emit_d(5, nc.gpsimd, nc.sync.dma_start)
                    emit_d(6, nc.vector, nc.gpsimd.dma_start)
                    emit_d(7, nc.vector, nc.scalar.dma_start)

    nc.compile()
    return nc


def _get():
    global _CACHED
    if _CACHED is None:
        _CACHED = _build()
    return _CACHED


def kernel(context, context_masks, query, query_masks, Wc, bc, Wq, bq, w_att, b_att):
    context = np.asarray(context, dtype=np.float32)
    context_masks = np.asarray(context_masks, dtype=np.float32)
    query = np.asarray(query, dtype=np.float32)
    query_masks = np.asarray(query_masks, dtype=np.float32)
    Wc = np.asarray(Wc, dtype=np.float32)
    bc = np.asarray(bc, dtype=np.float32)
    Wq = np.asarray(Wq, dtype=np.float32)
    bq = np.asarray(bq, dtype=np.float32)
    w_att = np.asarray(w_att, dtype=np.float32)
    # b_att shifts sim uniformly; softmax(axis=-1), max+softmax are invariant -> drop.

    def swz(mT):  # [H, N] -> [128, HT*N] fp16: row p holds blocks j = mT[j*128+p, :]
        n = mT.shape[1]
        return np.ascontiguousarray(
            mT.reshape(HT, 128, n).transpose(1, 0, 2).reshape(128, HT * n)
        ).astype(np.float16)

    def xtm(X):  # [C, H] -> [128p, CT, HT, 128q] fp16 tile-major X^T
        return np.ascontiguousarray(
            X.reshape(CT, 128, HT, 128).transpose(3, 0, 2, 1)).astype(np.float16)

    v = Wq.T @ (w_att * bc)   # r = Qm @ v (+ const, dropped: softmax-invariant)
    shared = {
        "wcT": swz(Wc.T),
        "wc": swz(Wc),
        "wqT": swz(Wq.T),
        "iden16": np.concatenate(
            [np.eye(128, dtype=np.float32), np.ones((128, 1), np.float32),
             np.ascontiguousarray(v.reshape(HT, 128).T)],
            axis=1).astype(np.float16),
        "ones32": np.ones((1, 128), np.float32),
    }
    in_maps = []
    for core in range(NC):
        g0 = core * BL
        cmT = (context_masks[g0:g0 + BL]
               .reshape(BL, CT, 128).transpose(2, 0, 1).reshape(128, BL * CT))
        cblob = np.concatenate([
            np.ascontiguousarray(w_att.reshape(HT, 128).T),
            cmT.astype(np.float32),
            np.ascontiguousarray(query_masks[g0:g0 + BL].T),
            np.broadcast_to(bc, (128, H)),
            np.ascontiguousarray((bq * w_att).reshape(HT, 128).T),
            np.ascontiguousarray(bq.reshape(HT, 128).T),
            np.ones((128, 1), np.float32),
        ], axis=1).astype(np.float32)
        in_maps.append({
            "xTm_in": np.stack([xtm(context[g0 + lb]) for lb in range(BL)]),
            "qT_in": np.concatenate([swz(query[g0 + lb].T) for lb in range(BL)],
                                    axis=1),
            "cblob": np.ascontiguousarray(cblob),
            **shared,
        })

    nc = _get()
    trace = os.environ.get("BASS_KERNEL_TRACE") == "1"
    res = run_bass_kernel_spmd(nc, in_maps, core_ids=list(range(NC)), trace=trace)
    if trace:
        global _LAST_RESULTS
        _LAST_RESULTS = res
        if res.exec_time_ns is not None:
            print(f"HW exec time: {res.exec_time_ns} ns")
        if res.instructions_and_trace is not None:
            print(f"trace: {res.instructions_and_trace[1]}")
    return np.concatenate(
        [res.results[i]["out"] for i in range(NC)], axis=0).astype(np.float32)


_LAST_RESULTS = None


if __name__ == "__main__":
    rng = np.random.default_rng(0)
    ins = {
        "context": rng.standard_normal((B, C, H), dtype=np.float32),
        "context_masks": np.ones((B, C), np.float32),
        "query": rng.standard_normal((B, Q, H), dtype=np.float32),
        "query_masks": np.ones((B, Q), np.float32),
        "Wc": (rng.random((H, H), dtype=np.float32) - 0.5) / 14.0,
        "bc": (rng.random(H, dtype=np.float32) - 0.5) / 14.0,
        "Wq": (rng.random((H, H), dtype=np.float32) - 0.5) / 14.0,
        "bq": (rng.random(H, dtype=np.float32) - 0.5) / 14.0,
        "w_att": (rng.random(H, dtype=np.float32) - 0.5) / 14.0,
        "b_att": np.float32(0.01),
    }
    out = kernel(**ins)
    print(out.shape, out.dtype)


# revision 58
# speedup vs baseline: 1.0150x; 1.0150x over previous
"""Trainium2 Bass kernel for BasicAttention (B=16, C=1024, Q=128, H=768).

Strategy
--------
Data-parallel over batch: 8 NeuronCores x 2 batches each. No collectives.

Per batch (X = context[b] [C,H], Qm = query[b] [Q,H]):
  qry   = Qm @ Wq^T + bq                      [Q,H]
  G     = (qry * w_att) @ Wc                  [Q,H]   (fused-projection trick)
  r     = Qm @ (Wq^T (w_att*bc))              [Q]     (const part softmax-invariant)
  sim   = X @ G^T + r (+ b_att, dropped: softmax/max-softmax shift-invariant)
  ctx   = X @ Wc^T + bc                       [C,H]
  alpha = softmax_q(sim);  a = (alpha*masks) @ qry
  beta  = softmax_c(max_q sim) * cmask;  b = beta @ ctx
  out   = [ctx, a, ctx*a, ctx*b]              [C,4H]

All big operands are fp16 on both sides: inputs are cast on the host and the
entire output is written fp16 and upcast to fp32 on the host after the
gather (output is 3/4 of HBM traffic; fp16 adds <=2^-11 absmax-rel error).
ctx lives in SBUF as fp16 so the c/d muls and a*ctx run in the DVE 16-bit
mode and the beta matmuls take fp16 operands directly.

exp(sim^T) comes straight out of the sim matmul via one ACT pass (r folded
in as the per-partition bias; |sim|<~3 so the softmax max-shift is
droppable) into fp16 expsimT [q,c] which feeds the a-matmul lhsT. Softmax
stats are fp16 PE transposes of expsimT tiles + DVE reductions.

The query-side projection is computed ONCE in transposed form (qryT[m] =
(Wq @ Qm^T)[m] via 36 128^3 matmuls); qwT and qryT+bq are two ACT passes
over the same PSUM tiles, and natural-layout qmm comes from 6 fp16 PE
transposes. r is a 6-step matvec against host-precomputed v = Wq^T(w_att*bc).

Scheduling is driven by the measured HW behavior:
 - HAM clock gate: PE defaults to 1.2 GHz and only reaches 2.4 GHz after
   ~3.4us of sustained activity -> a 10-matmul junk warm-up burst at t=0
   flips it early and the early real matmuls run 2x faster.
 - Each engine owns one hardware DGE queue (~110-160 GB/s each): inputs and
   outputs are spread across the sync/scalar/tensor/vector/gpsimd queues so
   no single ring serializes the traffic.
 - The last batch runs two-phase: all ctx tiles + sim/stats/beta first
   (PE-heavy), then a/c/d streamed per tile (DVE/ACT/DMA-heavy) so the
   d-quarter is not a serial tail.
"""

import os

import numpy as np

import concourse.bass as bass
import concourse.tile as tile
from concourse import bacc, bass_isa, mybir
from concourse.bass_utils import run_bass_kernel_spmd

F32 = mybir.dt.float32
F32R = mybir.dt.float32r
F16 = mybir.dt.float16
AX = mybir.AxisListType.X
EXP = mybir.ActivationFunctionType.Exp
IDENT = mybir.ActivationFunctionType.Identity

B, C, Q, H = 16, 1024, 128, 768
NC = 8
BL = B // NC          # batches per core
HT = H // 128         # 6 h-chunks
CT = C // 128         # 8 c-tiles
NSPLIT = ((0, 512), (512, 256))  # free-dim split respecting PSUM banks

_CACHED = None


def _build():
    nc = bacc.Bacc("TRN2", debug=False)

    # xTm[p, t, j, q] = X[t*128+q, j*128+p]: c-tile-major swizzled X^T so each
    # [128,128] (t,j) block is one matmul operand and quarters stream in early.
    xTm_in = nc.dram_tensor("xTm_in", (BL, 128, CT, HT, 128), F16, kind="ExternalInput")
    qT_in = nc.dram_tensor("qT_in", (128, BL * HT * Q), F16, kind="ExternalInput")
    wcT_d = nc.dram_tensor("wcT", (128, HT * H), F16, kind="ExternalInput")
    wc_d = nc.dram_tensor("wc", (128, HT * H), F16, kind="ExternalInput")
    wqT_d = nc.dram_tensor("wqT", (128, HT * H), F16, kind="ExternalInput")
    # const blob cols: wac[0:6] cm[6:22] qm[22:24] bcb[24:792] bqw[792:798]
    # bqT[798:804]
    cb_d = nc.dram_tensor("cblob", (128, 805), F32, kind="ExternalInput")
    # fp16 blob: identity[0:128] ones[128] vb[129:135]
    i16_d = nc.dram_tensor("iden16", (128, 135), F16, kind="ExternalInput")
    one_d = nc.dram_tensor("ones32", (1, 128), F32, kind="ExternalInput")
    out_d = nc.dram_tensor("out", (BL, C, 4 * H), F16, kind="ExternalOutput")

    with tile.TileContext(nc) as tc:
        with (
            tc.tile_pool(name="const", bufs=1) as cpool,
            tc.tile_pool(name="xt", bufs=2) as xtpool,
            tc.tile_pool(name="bigp", bufs=2) as bigpool,
            tc.tile_pool(name="qside", bufs=1) as qpool,
            tc.tile_pool(name="qside2", bufs=2) as q2pool,
            tc.tile_pool(name="ev", bufs=2) as evpool,
            tc.tile_pool(name="ev3", bufs=3) as ev3pool,
            tc.tile_pool(name="stat", bufs=1) as stpool,
            tc.tile_pool(name="ps768", bufs=2, space="PSUM") as ps768,
            tc.tile_pool(name="ps512", bufs=2, space="PSUM") as ps512,
            tc.tile_pool(name="pst", bufs=2, space="PSUM") as pst,
        ):
            # ---- constants / weights (once per core) ----
            wcT = cpool.tile([128, HT * H], F16, tag="wcT")   # block j: WcT[128j:128j+128, :]
            wqT = cpool.tile([128, HT * H], F16, tag="wqT")
            wcn = cpool.tile([128, HT * H], F16, tag="wcn")   # Wc natural (built on-chip)
            cb = cpool.tile([128, 805], F32, tag="cb")
            onesc = cb[:, 804:805]
            wac = cb[:, 0:6]
            cm = cb[:, 6:22]
            qm = cb[:, 22:24]
            bcb = cb[:, 24:24 + H]
            bqw = cb[:, 792:798]
            bqT = cb[:, 798:804]
            iden16 = cpool.tile([128, 135], F16, tag="iden16")
            i128 = iden16[:, 0:128]
            vb = iden16[:, 129:135]
            ones32 = cpool.tile([1, 128], F32R, tag="ones32")
            qT = {}
            xT = {}
            qTb = qpool.tile([128, BL * H], F16, tag="qTb")
            for lb in range(BL):
                qT[lb] = qTb[:, lb * H:(lb + 1) * H]
                xT[lb] = xtpool.tile([128, CT, HT, 128], F16, tag="xT", name=f"xT{lb}")

            # ---- input DMA: 2 HW DGE rings (sync/scalar, ~110-160 GB/s per
            # queue) carry the longest dependency chain's operands first
            # (wqT -> qpost -> G -> sim); the gpsimd software DGE (~160 GB/s,
            # measured) is a third queue for wcn/wcT_h1/batch-1 operands ----
            HH = HT * H // 2
            nc.sync.dma_start(iden16[:], i16_d.ap()[:, :])
            nc.sync.dma_start(ones32[:], one_d.ap()[:, :].bitcast(F32R))
            nc.sync.dma_start(wqT[:, HH:2 * HH], wqT_d.ap()[:, HH:2 * HH])
            nc.sync.dma_start(cb[:], cb_d.ap()[:, :])
            nc.sync.dma_start(xT[0][:, 0:2, :, :], xTm_in.ap()[0, :, 0:2, :, :])
            nc.sync.dma_start(xT[0][:, 4:6, :, :], xTm_in.ap()[0, :, 4:6, :, :])
            nc.sync.dma_start(xT[0][:, 6:8, :, :], xTm_in.ap()[0, :, 6:8, :, :])
            nc.scalar.dma_start(wqT[:, 0:HH], wqT_d.ap()[:, 0:HH])
            nc.scalar.dma_start(qTb[:, 0:H], qT_in.ap()[:, 0:H])
            nc.scalar.dma_start(wcT[:, 0:HH], wcT_d.ap()[:, 0:HH])
            nc.gpsimd.dma_start(wcn[:], wc_d.ap()[:, :])
            nc.gpsimd.dma_start(wcT[:, HH:2 * HH], wcT_d.ap()[:, HH:2 * HH])
            nc.gpsimd.dma_start(xT[0][:, 2:4, :, :], xTm_in.ap()[0, :, 2:4, :, :])
            nc.gpsimd.dma_start(qTb[:, H:2 * H], qT_in.ap()[:, H:2 * H])
            nc.gpsimd.dma_start(xT[1][:, 0:4, :, :], xTm_in.ap()[1, :, 0:4, :, :])
            nc.gpsimd.dma_start(xT[1][:, 4:8, :, :], xTm_in.ap()[1, :, 4:8, :, :])

            qmm = {}
            gT = {}
            r_sb = {}
            qwT = {}
            g_ps = {}

            # PE warm-up: dense junk matmuls while the first inputs stream in.
            # HAM needs ~3.4us of sustained PE activity to release the 1.2->2.4
            # GHz clock gate; this burst flips it before the real work starts.
            junk = qpool.tile([128, 512], F16, tag="junk")
            nc.gpsimd.memset(junk[:], 1.0)
            # NOTE: keep warm-up matmuls in pst with unique names -- tiles in
            # a shared-tag pool with no reader get dead-code-eliminated.
            for wi in range(24):
                jp = pst.tile([128, 512], F32, tag="tp", name=f"warm{wi}")
                nc.tensor.matmul(jp[:], junk[:, 0:128], junk[:], start=True, stop=True)

            def qpost(lb, split=False):
                # qryT[m-block] = (Wq @ Qm^T)[m] once; two ACT reads of the
                # same PSUM tile give qwT (scaled) and qnT (plain, +bq).
                # (One accumulation group per PSUM bank region at a time --
                # multiple open groups in one bank corrupt accumulation.)
                qwT[lb] = qpool.tile([128, H], F16, tag="qwT", name=f"qwT{lb}")
                qnT = q2pool.tile([128, H], F16, tag="qnT", name=f"qnT{lb}")

                # r[q] = (Qm @ v)[q], v = Wq^T (w_att*bc) precomputed on host
                r_ps = pst.tile([128, 1], F32, tag="tp", name=f"rps{lb}")
                for j in range(HT):
                    nc.tensor.matmul(r_ps[:], qT[lb][:, j * 128:(j + 1) * 128],
                                     vb[:, j:j + 1],
                                     start=(j == 0), stop=(j == HT - 1))
                r_sb[lb] = stpool.tile([128, 1], F32, tag=f"r_sb{lb}", name=f"r_sb{lb}")
                nc.scalar.copy(r_sb[lb][:], r_ps[:])

                if split:
                    # bridge the wqT-landing window with warm-keeper matmuls
                    for wi in range(8):
                        jp = pst.tile([128, 512], F32, tag="tp",
                                      name=f"warmq{wi}")
                        nc.tensor.matmul(jp[:], junk[:, 0:128], junk[:],
                                         start=True, stop=True)
                for m in range(HT):
                    qt_ps = pst.tile([128, 128], F32, tag="tp", name=f"qt{lb}{m}")
                    for j in range(HT):
                        nc.tensor.matmul(qt_ps[:],
                                         wqT[:, j * H + m * 128: j * H + (m + 1) * 128],
                                         qT[lb][:, j * 128:(j + 1) * 128],
                                         start=(j == 0), stop=(j == HT - 1))
                    nc.scalar.activation(qwT[lb][:, m * 128:(m + 1) * 128], qt_ps[:],
                                         IDENT, scale=wac[:, m:m + 1],
                                         bias=bqw[:, m:m + 1])
                    nc.scalar.activation(qnT[:, m * 128:(m + 1) * 128], qt_ps[:],
                                         IDENT, bias=bqT[:, m:m + 1])

                # natural-layout qmm = qry*qmask via fp16 PE transposes
                qmm[lb] = q2pool.tile([128, H], F16, tag="qmm", name=f"qmm{lb}")
                for m in range(HT):
                    tp = pst.tile([128, 128], F16, tag="tp", name=f"qx{lb}{m}")
                    nc.tensor.transpose(tp[:], qnT[:, m * 128:(m + 1) * 128], i128)
                    nc.vector.tensor_scalar_mul(qmm[lb][:, m * 128:(m + 1) * 128],
                                                tp[:], qm[:, lb:lb + 1])

            def G_mm(lb):
                g_ps[lb] = ps768.tile([128, H], F32, tag="mm768", name=f"g_ps{lb}")
                for j in range(HT):
                    for (n0, nw) in NSPLIT:
                        nc.tensor.matmul(g_ps[lb][:, n0:n0 + nw],
                                         qwT[lb][:, j * 128:(j + 1) * 128],
                                         wcn[:, j * H + n0: j * H + n0 + nw],
                                         start=(j == 0), stop=(j == HT - 1))

            def gpost(lb):
                g_sb = qpool.tile([128, H], F16, tag="g_sb", name=f"g_sb{lb}")
                nc.scalar.copy(g_sb[:], g_ps[lb][:])
                gT[lb] = q2pool.tile([128, H], F16, tag="gT", name=f"gT{lb}")
                for j in range(HT):
                    tp = pst.tile([128, 128], F16, tag="tp", name=f"tg{lb}{j}")
                    nc.tensor.transpose(tp[:], g_sb[:, j * 128:(j + 1) * 128], i128)
                    nc.vector.tensor_copy(gT[lb][:, j * 128:(j + 1) * 128], tp[:])

            # ---- context phases ----
            pending_d = []
            bb_sb = {}
            for lb in range(BL):
                ctx_all = bigpool.tile([128, CT * H], F16, tag="big", name=f"ctx{lb}")
                expsT = q2pool.tile([128, C], F16, tag="expsT", name=f"expsT{lb}")
                rsum = stpool.tile([128, CT], F32, tag=f"rsum{lb}", name=f"rsum{lb}")
                rcp = stpool.tile([128, CT], F32, tag=f"rcp{lb}", name=f"rcp{lb}")
                rscm = stpool.tile([128, CT], F32, tag=f"rscm{lb}", name=f"rscm{lb}")
                w8 = stpool.tile([128, CT], F32, tag=f"w8{lb}", name=f"w8{lb}")
                wm8 = stpool.tile([128, CT], F16, tag=f"wm8{lb}", name=f"wm8{lb}")
                b_acc = stpool.tile([1, H], F32R, tag=f"bacc{lb}", name=f"bacc{lb}")

                def simB(u, lb=lb, expsT=expsT):
                    """exp(sim^T) half u: [q, 512c] fp16, r added as ACT bias."""
                    st_ps = ps512.tile([128, 512], F32, tag="mm512")
                    for j in range(HT):
                        nc.tensor.matmul(st_ps[:],
                                         gT[lb][:, j * 128:(j + 1) * 128],
                                         xT[lb][:, u * 4:(u + 1) * 4, j, :],
                                         start=(j == 0), stop=(j == HT - 1))
                    nc.scalar.activation(expsT[:, u * 512:(u + 1) * 512], st_ps[:],
                                         EXP, bias=r_sb[lb][:])

                def stats(t, lb=lb, expsT=expsT, rsum=rsum, rcp=rcp, rscm=rscm,
                          w8=w8, wm8=wm8):
                    """fp16 transpose of an expsimT tile -> row stats on DVE."""
                    eT_ps = pst.tile([128, 128], F16, tag="tp", name=f"eT{lb}_{t}")
                    nc.tensor.transpose(eT_ps[:], expsT[:, t * 128:(t + 1) * 128],
                                        i128)
                    nc.vector.reduce_max(w8[:, t:t + 1], eT_ps[:], axis=AX)
                    nc.vector.reduce_sum(rsum[:, t:t + 1], eT_ps[:], axis=AX)
                    nc.vector.tensor_mul(wm8[:, t:t + 1], w8[:, t:t + 1],
                                         cm[:, lb * CT + t: lb * CT + t + 1])
                    nc.vector.reciprocal(rcp[:, t:t + 1], rsum[:, t:t + 1])
                    nc.scalar.mul(rscm[:, t:t + 1], rcp[:, t:t + 1],
                                  cm[:, lb * CT + t: lb * CT + t + 1])

                def ctx_mm(t, lb=lb, ctx_all=ctx_all):
                    cx_ps = ps768.tile([128, H], F32, tag="mm768")
                    for j in range(HT):
                        for (n0, nw) in NSPLIT:
                            nc.tensor.matmul(cx_ps[:, n0:n0 + nw],
                                             xT[lb][:, t, j, :],
                                             wcT[:, j * H + n0: j * H + n0 + nw],
                                             start=(j == 0), stop=(j == HT - 1))
                    nc.vector.tensor_add(ctx_all[:, t * H:(t + 1) * H], cx_ps[:], bcb[:])
                    nc.gpsimd.dma_start(out_d.ap()[lb, t * 128:(t + 1) * 128, 0:H],
                                        ctx_all[:, t * H:(t + 1) * H])

                def a_c(t, ceng=None, vnorm=False, lb=lb, ctx_all=ctx_all,
                        expsT=expsT, rscm=rscm):
                    a_ps = ps768.tile([128, H], F32, tag="mm768")
                    for (n0, nw) in NSPLIT:
                        nc.tensor.matmul(a_ps[:, n0:n0 + nw],
                                         expsT[:, t * 128:(t + 1) * 128],
                                         qmm[lb][:, n0:n0 + nw], start=True, stop=True)
                    stage = ev3pool.tile([128, 2 * H], F16, tag="stage")
                    if vnorm:
                        nc.vector.tensor_scalar_mul(stage[:, 0:H], a_ps[:],
                                                    rscm[:, t:t + 1])
                    else:
                        nc.scalar.mul(stage[:, 0:H], a_ps[:], rscm[:, t:t + 1])
                    (ceng or nc.vector).tensor_mul(
                        stage[:, H:2 * H], stage[:, 0:H],
                        ctx_all[:, t * H:(t + 1) * H])
                    (nc.sync if t % 2 == 0 else nc.scalar).dma_start(
                        out_d.ap()[lb, t * 128:(t + 1) * 128, H:3 * H], stage[:])

                def b_half(u, lb=lb, ctx_all=ctx_all, wm8=wm8, b_acc=b_acc):
                    """partial b = sum_c wm8[c]*ctx[c,:] over this half's 4 tiles"""
                    b5_ps = pst.tile([1, 512], F32, tag="tp", name=f"b5_{lb}{u}")
                    b2_ps = pst.tile([1, 256], F32, tag="tp", name=f"b2_{lb}{u}")
                    for tt in range(4):
                        t = u * 4 + tt
                        nc.tensor.matmul(b5_ps[:], wm8[:, t:t + 1],
                                         ctx_all[:, t * H: t * H + 512],
                                         start=(tt == 0), stop=(tt == 3))
                        nc.tensor.matmul(b2_ps[:], wm8[:, t:t + 1],
                                         ctx_all[:, t * H + 512: t * H + 768],
                                         start=(tt == 0), stop=(tt == 3))
                    if u == 0:
                        nc.vector.tensor_copy(b_acc[0:1, 0:512], b5_ps[:])
                        nc.vector.tensor_copy(b_acc[0:1, 512:H], b2_ps[:])
                    else:
                        nc.vector.tensor_add(b_acc[0:1, 0:512],
                                             b_acc[0:1, 0:512].bitcast(F32), b5_ps[:])
                        nc.vector.tensor_add(b_acc[0:1, 512:H],
                                             b_acc[0:1, 512:H].bitcast(F32), b2_ps[:])

                def beta_chain(lb=lb, w8=w8):
                    # 1/sum_c exp(q2c): free-axis partial, then the partition
                    # reduction + broadcast as two tiny PE matmuls (the gpsimd
                    # all-reduce sits in a busy FIFO and stalls the PE here)
                    sp = stpool.tile([128, 1], F32, tag=f"sp{lb}", name=f"sp{lb}")
                    nc.vector.reduce_sum(sp[:], w8[:, 0:CT], axis=AX)
                    sp16 = stpool.tile([128, 1], F16, tag=f"sp6{lb}",
                                       name=f"sp6{lb}")
                    nc.vector.tensor_copy(sp16[:], sp[:])
                    tot_ps = pst.tile([1, 1], F32, tag="tp", name=f"tot{lb}")
                    nc.tensor.matmul(tot_ps[:], sp16[:], iden16[:, 128:129],
                                     start=True, stop=True)
                    tot_sb = stpool.tile([1, 1], F16, tag=f"tot{lb}",
                                         name=f"tots{lb}")
                    nc.vector.tensor_copy(tot_sb[:], tot_ps[:])
                    spa_ps = pst.tile([128, 1], F32, tag="tp", name=f"spa{lb}")
                    nc.tensor.matmul(spa_ps[:], junk[0:1, 0:128], tot_sb[:],
                                     start=True, stop=True)
                    rs1 = stpool.tile([128, 1], F32, tag=f"rs1{lb}", name=f"rs1{lb}")
                    nc.vector.reciprocal(rs1[:], spa_ps[:])
                    return rs1

                def bb_chain(rs1, lb=lb, b_acc=b_acc):
                    # broadcast b to 128 partitions via K=1 ones outer product,
                    # folding the beta normalization into the PSUM->SBUF copy
                    bb_ps = ps768.tile([128, H], F32, tag="mm768", name=f"bb_ps{lb}")
                    for (n0, nw) in NSPLIT:
                        nc.tensor.matmul(bb_ps[:, n0:n0 + nw], ones32[:],
                                         b_acc[0:1, n0:n0 + nw], start=True, stop=True)
                    bb = evpool.tile([128, H], F16, tag="bb")
                    nc.scalar.mul(bb[:], bb_ps[:], rs1[:, 0:1])
                    return bb

                def emit_d(t, eng, ring, lb=lb, ctx_all=ctx_all):
                    d_sb = ev3pool.tile([128, H], F16, tag=("d_sb", "c_sb")[t % 2],
                                        name=f"d{lb}_{t}")
                    eng.tensor_mul(d_sb[:], ctx_all[:, t * H:(t + 1) * H],
                                   bb_sb[lb][:])
                    ring(out_d.ap()[lb, t * 128:(t + 1) * 128, 3 * H:4 * H], d_sb[:])

                def pop():
                    if pending_d:
                        pending_d.pop(0)()

                if lb == 0:
                    qpost(0, split=True)
                    G_mm(0)
                    gpost(0)
                    simB(0)
                    ctx_mm(0)
                    stats(0)
                    a_c(0)
                    ctx_mm(1)
                    stats(1)
                    a_c(1)
                    ctx_mm(2)
                    stats(2)
                    a_c(2)
                    ctx_mm(3)
                    stats(3)
                    a_c(3)
                    b_half(0)
                    simB(1)
                    stats(4)
                    ctx_mm(4)
                    a_c(4)
                    qpost(1)
                    stats(5)
                    ctx_mm(5)
                    a_c(5)
                    G_mm(1)
                    gpost(1)
                    stats(6)
                    ctx_mm(6)
                    a_c(6)
                    stats(CT - 1)
                    rs1 = beta_chain()
                    ctx_mm(CT - 1)
                    a_c(CT - 1)
                    b_half(1)
                    bb_sb[lb] = bb_chain(rs1)
                    pending_d = [
                        (lambda t=t, f=emit_d: f(t, nc.vector,
                                                 nc.gpsimd.dma_start))
                        for t in range(CT)
                    ]
                else:
                    # phase A: everything beta/sim/ctx (PE-heavy), lb=0's
                    # leftover d-quarters popped in to fill DVE/DMA slack
                    # (qpost/G/gpost for this batch were hoisted into lb=0)
                    simB(0)
                    ctx_mm(0)
                    pop()
                    ctx_mm(1)
                    pop()
                    ctx_mm(2)
                    pop()
                    stats(0)
                    ctx_mm(3)
                    pop()
                    stats(1)
                    a_c(0)
                    stats(2)
                    a_c(1, vnorm=True)
                    stats(3)
                    ctx_mm(4)
                    pop()
                    b_half(0)
                    simB(1)
                    a_c(2)
                    ctx_mm(5)
                    pop()
                    stats(4)
                    a_c(3, vnorm=True)
                    ctx_mm(6)
                    pop()
                    stats(5)
                    stats(6)
                    ctx_mm(7)
                    pop()
                    stats(7)
                    rs1 = beta_chain()
                    b_half(1)
                    # phase B: the remaining a/c plus the d stream (DVE/ACT/
                    # DMA-heavy); the first a_c's overlap the beta/bb chain.
                    a_c(4)
                    pop()
                    a_c(5)
                    bb_sb[lb] = bb_chain(rs1)
                    a_c(6)
                    emit_d(0, nc.vector, nc.gpsimd.dma_start)
                    a_c(7)
                    emit_d(1, nc.vector, nc.sync.dma_start)
                    emit_d(2, nc.gpsimd, nc.gpsimd.dma_start)
                    emit_d(3, nc.vector, nc.sync.dma_start)
                    emit_d(4, nc.vector, nc.gpsimd.dma_start)
                    emit_d(5, nc.gpsimd, nc.sync.dma_start)
                    emit_d(6, nc.vector, nc.gpsimd.dma_start)
                    emit_d(7, nc.vector, nc.scalar.dma_start)

    nc.compile()
    return nc


def _get():
    global _CACHED
    if _CACHED is None:
        _CACHED = _build()
    return _CACHED


def kernel(context, context_masks, query, query_masks, Wc, bc, Wq, bq, w_att, b_att):
    context = np.asarray(context, dtype=np.float32)
    context_masks = np.asarray(context_masks, dtype=np.float32)
    query = np.asarray(query, dtype=np.float32)
    query_masks = np.asarray(query_masks, dtype=np.float32)
    Wc = np.asarray(Wc, dtype=np.float32)
    bc = np.asarray(bc, dtype=np.float32)
    Wq = np.asarray(Wq, dtype=np.float32)
    bq = np.asarray(bq, dtype=np.float32)
    w_att = np.asarray(w_att, dtype=np.float32)
    # b_att shifts sim uniformly; softmax(axis=-1), max+softmax are invariant -> drop.

    def swz(mT):  # [H, N] -> [128, HT*N] fp16: row p holds blocks j = mT[j*128+p, :]
        n = mT.shape[1]
        return np.ascontiguousarray(
            mT.reshape(HT, 128, n).transpose(1, 0, 2).reshape(128, HT * n)
        ).astype(np.float16)

    def xtm(X):  # [C, H] -> [128p, CT, HT, 128q] fp16 tile-major X^T
        return np.ascontiguousarray(
            X.reshape(CT, 128, HT, 128).transpose(3, 0, 2, 1)).astype(np.float16)

    v = Wq.T @ (w_att * bc)   # r = Qm @ v (+ const, dropped: softmax-invariant)
    shared = {
        "wcT": swz(Wc.T),
        "wc": swz(Wc),
        "wqT": swz(Wq.T),
        "iden16": np.concatenate(
            [np.eye(128, dtype=np.float32), np.ones((128, 1), np.float32),
             np.ascontiguousarray(v.reshape(HT, 128).T)],
            axis=1).astype(np.float16),
        "ones32": np.ones((1, 128), np.float32),
    }
    in_maps = []
    for core in range(NC):
        g0 = core * BL
        cmT = (context_masks[g0:g0 + BL]
               .reshape(BL, CT, 128).transpose(2, 0, 1).reshape(128, BL * CT))
        cblob = np.concatenate([
            np.ascontiguousarray(w_att.reshape(HT, 128).T),
            cmT.astype(np.float32),
            np.ascontiguousarray(query_masks[g0:g0 + BL].T),
            np.broadcast_to(bc, (128, H)),
            np.ascontiguousarray((bq * w_att).reshape(HT, 128).T),
            np.ascontiguousarray(bq.reshape(HT, 128).T),
            np.ones((128, 1), np.float32),
        ], axis=1).astype(np.float32)
        in_maps.append({
            "xTm_in": np.stack([xtm(context[g0 + lb]) for lb in range(BL)]),
            "qT_in": np.concatenate([swz(query[g0 + lb].T) for lb in range(BL)],
                                    axis=1),
            "cblob": np.ascontiguousarray(cblob),
            **shared,
        })

    nc = _get()
    trace = os.environ.get("BASS_KERNEL_TRACE") == "1"
    res = run_bass_kernel_spmd(nc, in_maps, core_ids=list(range(NC)), trace=trace)
    if trace:
        global _LAST_RESULTS
        _LAST_RESULTS = res
        if res.exec_time_ns is not None:
            print(f"HW exec time: {res.exec_time_ns} ns")
        if res.instructions_and_trace is not None:
            print(f"trace: {res.instructions_and_trace[1]}")
    return np.concatenate(
        [res.results[i]["out"] for i in range(NC)], axis=0).astype(np.float32)


_LAST_RESULTS = None


if __name__ == "__main__":
    rng = np.random.default_rng(0)
    ins = {
        "context": rng.standard_normal((B, C, H), dtype=np.float32),
        "context_masks": np.ones((B, C), np.float32),
        "query": rng.standard_normal((B, Q, H), dtype=np.float32),
        "query_masks": np.ones((B, Q), np.float32),
        "Wc": (rng.random((H, H), dtype=np.float32) - 0.5) / 14.0,
        "bc": (rng.random(H, dtype=np.float32) - 0.5) / 14.0,
        "Wq": (rng.random((H, H), dtype=np.float32) - 0.5) / 14.0,
        "bq": (rng.random(H, dtype=np.float32) - 0.5) / 14.0,
        "w_att": (rng.random(H, dtype=np.float32) - 0.5) / 14.0,
        "b_att": np.float32(0.01),
    }
    out = kernel(**ins)
    print(out.shape, out.dtype)
